# revision 8
# baseline (speedup 1.0000x reference)
"""FKANLinear fused kernel for 8 TRN2 NeuronCores (v2).

Differences from v1 (kernel.py):
  - PSUM restructure: per 512-col block, one accumulation chain per round.
    Round 1 (static coefs: fourier c2..c9, wavelet c10/c11) -> spill PSUM->SBUF
    via DMA.  Round 2 (post-collective: mono c0/c1 with dynamic remix, rho
    c12..c15) re-injects the spill with an identity matmul and adds the bias
    via an ones-plane in chunk 15; result DMAs straight PSUM->DRAM.
  - No m0 tile: chunk 0 runs as two half-height matmuls reading xd (x) and
    ssq (x^2) directly.
  - Bias via CO folded into chunk-15 lower-half coefficients (ones plane).
  - Engine rebalance: ACT/DVE/Pool each ~1/3 of elementwise work; rho relu
    on ACT (per-partition bias) / Pool split.
  - Optional bf16 planes (BF16=True) for 2x DVE throughput on mult chains.
"""

import sys
import numpy as np

if "/opt/trn_rl_repo" not in sys.path:
    sys.path.insert(0, "/opt/trn_rl_repo")

N_CORES = 8
B, IN, OUT = 32768, 64, 32
BS = B // N_CORES          # 4096 rows per core
G, P = 8, 3
TAY = 4
JDEG, JA, JB = 4, 1.0, 1.0
CDEG = 4
FREQ = 8
WCH = 4
TEMP = 2.0

F32 = np.float32


# ----------------------------------------------------------------------------
# host-side folding (shared with v1, CO/RW layouts changed)
# ----------------------------------------------------------------------------

def _softplus(z):
    z = np.asarray(z, np.float64)
    return np.log1p(np.exp(-np.abs(z))) + np.maximum(z, 0.0)


def _softmax(z, axis):
    z = np.asarray(z, np.float64)
    m = z.max(axis=axis, keepdims=True)
    e = np.exp(z - m)
    return e / e.sum(axis=axis, keepdims=True)


def _jacobi_mono():
    a, b = JA, JB
    terms = np.zeros((JDEG + 1, 5))
    terms[0, 0] = 1.0
    if JDEG >= 1:
        terms[1, 1] = 0.5 * 2.0 * (a + 1.0) / np.sqrt(2.0)
        terms[1, 0] = 0.5 * (a - b) / np.sqrt(2.0)
    for n in range(2, JDEG + 1):
        k = n - 1
        A1 = 2 * k + a + b
        A2 = 2 * (k + 1) * (k + a + b + 1) * (A1 + 1)
        A4 = 2 * (k + a) * (k + b) * (A1 + 2)
        c_x = (A1 + 1) * (A1 + 2) * A1 / A2
        c_0 = (A1 + 1) * (a * a - b * b) / A2
        Jn = np.zeros(5)
        Jn[1:] += c_x * terms[n - 1][:4]
        Jn += c_0 * terms[n - 1]
        Jn -= (A4 / A2) * terms[n - 2]
        terms[n] = Jn / np.sqrt(n + 1.0)
    return terms


def _cheby_mono():
    T = np.zeros((CDEG + 1, 5))
    T[0, 0] = 1.0
    T[1, 1] = 1.0
    for n in range(2, CDEG + 1):
        shift = np.zeros(5)
        shift[1:] = T[n - 1][:4]
        T[n] = 2.0 * shift - T[n - 2]
    norm = 1.0 / np.sqrt(np.arange(CDEG + 1) + 1.0)
    return T * norm[:, None]


def _bspline_tspace_phi(t):
    grid = np.concatenate([np.zeros(3), np.linspace(0.0, 8.0, G + 1), np.full(3, 8.0)])
    te = t[:, None]
    bases = ((te >= grid[None, :-1]) & (te < grid[None, 1:])).astype(np.float64)
    mask_last = t == grid[-1]
    bases[mask_last, :] = 0.0
    bases[mask_last, -1] = 1.0
    for r in range(1, P + 1):
        ld = np.maximum(grid[r:-1] - grid[:-(r + 1)], 1e-12)
        rd = np.maximum(grid[r + 1:] - grid[1:-r], 1e-12)
        left = (te - grid[None, :-(r + 1)]) / ld[None, :] * bases[:, :-1]
        right = (grid[None, r + 1:] - te) / rd[None, :] * bases[:, 1:]
        bases = left + right
    return bases


def _bspline_truncpow_matrix():
    S = 6000
    t = np.linspace(0.0, 8.0, S)
    t = t + 1e-7
    t = np.clip(t, 0.0, 8.0)
    phi = _bspline_tspace_phi(t)
    Fm = np.zeros((S, 11))
    Fm[:, 0] = 1.0
    Fm[:, 1] = t
    Fm[:, 2] = t * t
    Fm[:, 3] = t ** 3
    for j in range(1, 8):
        Fm[:, 3 + j] = np.maximum(t - j, 0.0) ** 3
    M, _, _, _ = np.linalg.lstsq(Fm, phi, rcond=None)
    return M


def fold_constants(inputs):
    x = inputs["x"]
    base_v = np.asarray(inputs["base_v"], np.float64)
    base_g = np.asarray(inputs["base_g"], np.float64)
    base_bias = np.asarray(inputs["base_bias"], np.float64)
    gains = np.asarray(inputs["gains"], np.float64)
    alpha = float(_softplus(inputs["alpha_logit"]))
    beta = float(_softplus(inputs["beta_logit"]))
    mixw = _softmax(np.asarray(inputs["mix_logits"], np.float64) / TEMP, axis=-1)
    sg = _softplus(gains)

    def ceff(name, f):
        return np.asarray(inputs[name], np.float64) * mixw[..., f:f + 1] * sg[f] * beta

    C_bs = ceff("bspline_coef", 0)
    C_ty = ceff("taylor_coef", 1)
    C_jb = ceff("jacobi_coef", 2)
    C_cb = ceff("cheby_coef", 3)
    C_fr = ceff("fourier_coef", 4)
    C_wv = ceff("wavelet_coef", 5)

    vn = np.sqrt((base_v ** 2).sum(axis=1, keepdims=True))
    Walpha = alpha * base_g * base_v / vn              # (32, 64)
    bias_alpha = alpha * base_bias                      # (32,)

    mono = np.zeros((OUT, IN, 5))
    fac = np.array([1.0, 1.0, 2.0, 6.0])
    mono[:, :, :4] += C_ty / fac[None, None, :]
    mono += np.einsum("oin,nd->oid", C_jb, _jacobi_mono())
    mono += np.einsum("oin,nd->oid", C_cb, _cheby_mono())

    fnorm = 1.0 / np.sqrt(2.0 * FREQ)
    Ccos = C_fr[:, :, :FREQ] * fnorm
    Csin = C_fr[:, :, FREQ:] * fnorm
    Tc = np.zeros((9, 9)); Tc[0, 0] = 1.0; Tc[1, 1] = 1.0
    Uc = np.zeros((9, 9)); Uc[0, 0] = 1.0; Uc[1, 1] = 2.0
    for n in range(2, 9):
        for M_ in (Tc, Uc):
            sh = np.zeros(9); sh[1:] = M_[n - 1][:8]
            M_[n] = 2.0 * sh - M_[n - 2]
    Ccpow = np.einsum("oik,kj->oij", Ccos, Tc[1:9, :])   # (32,64,9)
    Cspow = np.einsum("oik,kj->oij", Csin, Uc[0:8, :])   # (32,64,9)

    a_w = _softplus(np.asarray(inputs["wavelet_scale_logit"], np.float64)) + 1e-6
    inva = 1.0 / a_w
    nshia = -np.asarray(inputs["wavelet_shift"], np.float64) * inva

    M = _bspline_truncpow_matrix()
    CF = np.einsum("oik,mk->oim", C_bs, M)               # (32,64,11)
    P_poly = CF[:, :, :4]
    R_rho = CF[:, :, 4:]                                 # (32,64,7)

    CW = np.zeros((128, 16 * OUT))

    def put(c, half, arr_oi):
        CW[half * 64:(half + 1) * 64, 32 * c:32 * (c + 1)] = arr_oi.T

    put(0, 0, Walpha + mono[:, :, 1])
    put(0, 1, mono[:, :, 2])
    put(1, 0, mono[:, :, 3])
    put(1, 1, mono[:, :, 4])
    for j in range(1, 9):
        put(1 + j, 0, Ccpow[:, :, j])
        put(1 + j, 1, Cspow[:, :, j - 1])
    put(10, 0, C_wv[:, :, 0]); put(10, 1, C_wv[:, :, 1])
    put(11, 0, C_wv[:, :, 2]); put(11, 1, C_wv[:, :, 3])
    # c12..c15 filled on device (rho coefs; c15 bottom = CO + cd0)

    # static const coefs, on rows 64:128 (feed the ones plane of chunk 15)
    CO = np.zeros((128, OUT))
    CO[64:128, :] = (mono[:, :, 0] + Ccpow[:, :, 0]).T
    CO[64, :] += bias_alpha

    PW = np.zeros((128, 4 * OUT))
    for d in range(4):
        PW[0:64, 32 * d:32 * (d + 1)] = P_poly[:, :, d].T
    PW[64:128] = PW[0:64]

    # rho coefs paired per chunk: RW4[:, q] = [R(2q) ; R(2q+1)], q=3: [R6; 0]
    RW4 = np.zeros((128, 4 * OUT))
    for q in range(4):
        RW4[0:64, 32 * q:32 * (q + 1)] = R_rho[:, :, 2 * q].T
        if 2 * q + 1 < 7:
            RW4[64:128, 32 * q:32 * (q + 1)] = R_rho[:, :, 2 * q + 1].T

    WVP = np.zeros((128, 4))
    for p in range(2):
        WVP[0:64, 2 * p] = inva[:, 2 * p]
        WVP[64:128, 2 * p] = inva[:, 2 * p + 1]
        WVP[0:64, 2 * p + 1] = nshia[:, 2 * p]
        WVP[64:128, 2 * p + 1] = nshia[:, 2 * p + 1]

    ID = np.eye(128)

    return {
        "CW": CW.astype(F32), "CO": CO.astype(F32), "PW": PW.astype(F32),
        "RW4": RW4.astype(F32), "WVP": WVP.astype(F32), "ID": ID.astype(F32),
        "ID32": np.eye(OUT).astype(F32),
    }


# ----------------------------------------------------------------------------
# numpy emulation of the device algorithm (validates folding + structure)
# ----------------------------------------------------------------------------

def _rnd(x, bf16):
    if not bf16:
        return np.asarray(x, np.float32)
    import ml_dtypes
    return np.asarray(x, ml_dtypes.bfloat16).astype(np.float32)


def numpy_forward(inputs, bf16=False):
    consts = fold_constants(inputs)
    x = np.asarray(inputs["x"], np.float64)
    CW = consts["CW"].astype(np.float64)
    CO = consts["CO"].astype(np.float64)
    PW = consts["PW"].astype(np.float64)
    RW4 = consts["RW4"].astype(np.float64)
    WVP = consts["WVP"].astype(np.float64)

    xmin = x.min(axis=0); xmax = x.max(axis=0)
    pad = (xmax - xmin) < 1e-8
    xmin = np.where(pad, xmin - 0.5, xmin)
    xmax = np.where(pad, xmax + 0.5, xmax)
    rng = xmax - xmin
    b = 8.0 / rng
    a = -xmin * b
    P_poly = np.stack([PW[0:64, 32 * d:32 * (d + 1)] for d in range(4)], axis=-1)
    binom = {(0, 0): 1, (1, 0): 1, (1, 1): 1, (2, 0): 1, (2, 1): 2, (2, 2): 1,
             (3, 0): 1, (3, 1): 3, (3, 2): 3, (3, 3): 1}
    Cdyn = np.zeros((IN, OUT, 4))
    for d in range(4):
        for e in range(d + 1):
            Cdyn[:, :, e] += P_poly[:, :, d] * (binom[(d, e)] * a ** (d - e) * b ** e)[:, None]
    CW = CW.copy()
    CW[0:64, 0:32] += Cdyn[:, :, 1]
    CW[64:128, 0:32] += Cdyn[:, :, 2]
    CW[0:64, 32:64] += Cdyn[:, :, 3]
    # c12..c15 coefs
    b3 = np.concatenate([b ** 3, b ** 3])               # (128,)
    for q in range(4):
        CW[:, 32 * (12 + q):32 * (13 + q)] = RW4[:, 32 * q:32 * (q + 1)] * b3[:, None]
    CW[64:128, 32 * 15:32 * 16] = CO[64:128] + Cdyn[:, :, 0]

    Bn = x.shape[0]
    kap = [xmin + j * rng / 8.0 for j in range(1, 8)]

    sh = np.sin(0.5 * x); sq2 = sh * sh
    sh4 = np.sin(0.25 * x); sq4 = sh4 * sh4
    cosx = 1.0 - 2.0 * sq2
    ch2 = 1.0 - 2.0 * sq4
    sinx = 2.0 * sh * ch2
    ssq = x * x
    x3 = ssq * x; x4 = ssq * ssq

    y = np.zeros((Bn, OUT))

    def mm(cols, plane, rows):
        nonlocal y
        pl = _rnd(plane, bf16) if rows is None else _rnd(plane, bf16)
        lo, hi = rows
        w = CW[lo:hi, cols]
        y = y + _rnd(pl, False) @ w

    # round 1: fourier c2..c9 (bf16-able), wavelet c10/c11
    pk = np.stack([cosx, sinx], axis=1).reshape(Bn, -1)  # emulate [c; s] halves
    # simpler: build planes explicitly per chunk
    cc, ssn = cosx, sinx
    for j in range(1, 9):
        top = _rnd(cc ** j, bf16)
        bot = _rnd(ssn * cc ** (j - 1), bf16)
        w_top = CW[0:64, 32 * (1 + j):32 * (2 + j)]
        w_bot = CW[64:128, 32 * (1 + j):32 * (2 + j)]
        y = y + top @ w_top + bot @ w_bot
    for p in range(2):
        u0 = x * WVP[None, 0:64, 2 * p] + WVP[None, 0:64, 2 * p + 1]
        u1 = x * WVP[None, 64:128, 2 * p] + WVP[None, 64:128, 2 * p + 1]
        f0 = _rnd((u0 ** 2 - 1) * np.exp(-0.5 * u0 ** 2), bf16)
        f1 = _rnd((u1 ** 2 - 1) * np.exp(-0.5 * u1 ** 2), bf16)
        y = y + f0 @ CW[0:64, 32 * (10 + p):32 * (11 + p)]
        y = y + f1 @ CW[64:128, 32 * (10 + p):32 * (11 + p)]
    # round 2: mono (f32), rho (bf16-able), ones/CO
    y = y + x @ CW[0:64, 0:32] + ssq @ CW[64:128, 0:32]
    y = y + x3 @ CW[0:64, 32:64] + x4 @ CW[64:128, 32:64]
    for q in range(4):
        j0 = 2 * q
        top = _rnd(np.maximum(x - kap[j0][None, :], 0.0) ** 3, bf16)
        y = y + top @ CW[0:64, 32 * (12 + q):32 * (13 + q)]
        if j0 + 1 < 7:
            bot = _rnd(np.maximum(x - kap[j0 + 1][None, :], 0.0) ** 3, bf16)
            y = y + bot @ CW[64:128, 32 * (12 + q):32 * (13 + q)]
    y = y + np.ones((Bn, 64)) @ CW[64:128, 32 * 15:32 * 16]
    return y.astype(F32)


# ----------------------------------------------------------------------------
# device kernel
# ----------------------------------------------------------------------------

SBC = 1024                  # elementwise superblock columns
BF16 = True
BUILD_KW = {"bf16": BF16}


def build_nc(debug=False, reps=1, no_collective=False, sbc=SBC, bf16=BF16,
             ablate=0):
    import concourse.bass as bass
    import concourse.bacc as bacc
    import concourse.mybir as mybir
    import concourse.tile as tile

    dt = mybir.dt.float32
    dtr = mybir.dt.float32r
    dtb = mybir.dt.bfloat16
    dtp = dtb if bf16 else dtr          # plane dtype for matmul rhs
    AF = mybir.ActivationFunctionType
    ALU = mybir.AluOpType
    AX = mybir.AxisListType

    NSUP = BS // sbc
    GRP = 512
    NGS = sbc // GRP            # groups per superblock
    KCOL = 512
    NKS = sbc // KCOL           # col-blocks per superblock

    def f32r(ap):
        return ap if ap.dtype in (dtr, dtb) else ap.bitcast(dtr)

    nc = bacc.Bacc("TRN2", target_bir_lowering=False, debug=False,
                   enable_asserts=True, num_devices=N_CORES)

    xs = nc.dram_tensor("xs", [BS, IN], dt, kind="ExternalInput").ap()
    cw_d = nc.dram_tensor("CW", [128, 16 * OUT], dtr, kind="ExternalInput").ap()
    co_d = nc.dram_tensor("CO", [128, OUT], dt, kind="ExternalInput").ap()
    pw_d = nc.dram_tensor("PW", [128, 4 * OUT], dt, kind="ExternalInput").ap()
    rw_d = nc.dram_tensor("RW4", [128, 4 * OUT], dt, kind="ExternalInput").ap()
    wv_d = nc.dram_tensor("WVP", [128, 4], dt, kind="ExternalInput").ap()
    id_d = nc.dram_tensor("ID", [128, 128], dt, kind="ExternalInput").ap()
    id32_d = nc.dram_tensor("ID32", [OUT, OUT], dtr, kind="ExternalInput").ap()
    y_d = nc.dram_tensor("y", [OUT, BS], dt, kind="ExternalOutput").ap()

    with tile.TileContext(nc) as tc:
        with (
            tc.tile_pool(name="const", bufs=1) as cpool,
            tc.tile_pool(name="sb", bufs=2) as sb,
            tc.tile_pool(name="fourp", bufs=3) as fourp,
            tc.tile_pool(name="xpipe", bufs=3) as xpipe,
            tc.tile_pool(name="pers", bufs=1) as pers,
            tc.tile_pool(name="tp", bufs=2, space="PSUM") as tps,
            tc.tile_pool(name="acc", bufs=4, space="PSUM") as accp,
            tc.tile_pool(name="dram", bufs=1, space="DRAM") as dram,
        ):
            # constants
            cwt = cpool.tile([128, 16 * OUT], dtr, tag="cwt")
            cot = cpool.tile([128, OUT], dt, tag="cot")
            pwt = cpool.tile([128, 4 * OUT], dt, tag="pwt")
            rwt = cpool.tile([128, 4 * OUT], dt, tag="rwt")
            wvt = cpool.tile([128, 4], dt, tag="wvt")
            idt = cpool.tile([128, 128], dt, tag="idt")
            id32 = cpool.tile([OUT, OUT], dtr, tag="id32")
            nc.sync.dma_start(out=cwt[:, :], in_=cw_d[:, :])
            nc.sync.dma_start(out=cot[:, :], in_=co_d[:, :])
            nc.sync.dma_start(out=pwt[:, :], in_=pw_d[:, :])
            nc.sync.dma_start(out=rwt[:, :], in_=rw_d[:, :])
            nc.sync.dma_start(out=wvt[:, :], in_=wv_d[:, :])
            nc.sync.dma_start(out=idt[:, :], in_=id_d[:, :])
            nc.sync.dma_start(out=id32[:, :], in_=id32_d[:, :])

            # bf16 coef tile for bf16 plane chunks (c2..c15)
            if bf16:
                cwb = cpool.tile([128, 16 * OUT], dtb, tag="cwb")
                nc.vector.tensor_copy(out=cwb[:, 64:384],
                                      in_=cwt[:, 64:384].bitcast(dt))

            def lhs(c0, c1=None, rows=(0, 128)):
                """matmul lhsT slice for chunk c0 (cols 32c0..32c0+32)."""
                lo, hi = rows
                t = cwb if (bf16 and c0 >= 2) else cwt
                return f32r(t[lo:hi, 32 * c0:32 * (c0 + 1)])

            # persistent rho7/ones planes per superblock (bottom = 1.0)
            rho7s = []
            ones_sc = cpool.tile([128, sbc], dt, tag="ones_sc")
            nc.vector.memset(ones_sc[64:128, :], 1.0)
            for s in range(NSUP):
                r7 = pers.tile([128, sbc], dtp, tag=f"rho7_{s}")
                if bf16:
                    nc.vector.memset(r7[64:128, :], 1.0)
                else:
                    nc.scalar.copy(r7[64:128, :], ones_sc[64:128, :])
                rho7s.append(r7)

            for _rep in range(reps):
                mm = pers.tile([128, 2 * NSUP], dt, tag="mm", bufs=2)
                xds = []

                # ---------------- phase A: load, transpose, min/max --------
                for s in range(NSUP):
                    xd = pers.tile([128, sbc], dtr, tag=f"xd{s}", bufs=2)
                    xds.append(xd)
                    for g in range(NGS):
                        gg = s * NGS + g
                        base = s * sbc + g * GRP
                        xt8 = xpipe.tile([128, 8 * IN], dt, tag="xin")
                        src = xs[base:base + GRP, :].rearrange(
                            "(t p) i -> p t i", p=128)
                        dst = xt8.rearrange("p (t i) -> p t i", i=2 * IN)
                        nc.sync.dma_start(out=dst[:, :, 0:IN], in_=src)
                        nc.sync.dma_start(out=dst[:, :, IN:2 * IN], in_=src)
                        tp = tps.tile([128, 512], dt, tag="tp")
                        for t in range(4):
                            nc.tensor.transpose(
                                tp[:, 128 * t:128 * (t + 1)],
                                xt8[:, t * 128:(t + 1) * 128], idt[:, :])
                        eng = ((nc.scalar, nc.vector)[gg % 2] if bf16 else
                               (nc.scalar, nc.vector)[gg % 2])
                        if eng is nc.scalar:
                            eng.copy(xd[:, g * GRP:(g + 1) * GRP], tp[:, :])
                        else:
                            eng.tensor_copy(out=xd[:, g * GRP:(g + 1) * GRP],
                                            in_=tp[:, :])
                    nc.vector.tensor_reduce(out=mm[:, s:s + 1], in_=xd[:, :],
                                            axis=AX.X, op=ALU.min)
                    nc.vector.tensor_reduce(out=mm[:, NSUP + s:NSUP + s + 1],
                                            in_=xd[:, :], axis=AX.X, op=ALU.max)

                locmin = pers.tile([128, 1], dt, tag="locmin")
                locmax = pers.tile([128, 1], dt, tag="locmax")
                nc.vector.tensor_reduce(out=locmin[:, :], in_=mm[:, 0:NSUP],
                                        axis=AX.X, op=ALU.min)
                nc.vector.tensor_reduce(out=locmax[:, :], in_=mm[:, NSUP:2 * NSUP],
                                        axis=AX.X, op=ALU.max)

                # ---------------- collective: allgather min/max ------------
                bounce_in = dram.tile([2, IN], dt, tag="cin")
                bounce_out = dram.tile([2 * N_CORES, IN], dt, tag="cout")
                nc.sync.dma_start(out=bounce_in[0:1, :], in_=locmin[0:IN, :])
                nc.sync.dma_start(out=bounce_in[1:2, :], in_=locmax[0:IN, :])
                if no_collective:
                    for r in range(N_CORES):
                        nc.gpsimd.dma_start(out=bounce_out[2 * r:2 * r + 2, :],
                                            in_=bounce_in[:, :])
                else:
                    nc.gpsimd.collective_compute(
                        "AllGather", mybir.AluOpType.bypass,
                        replica_groups=[list(range(N_CORES))],
                        ins=[bounce_in.opt()],
                        outs=[bounce_out.opt()],
                    )
                gm = pers.tile([128, 2 * N_CORES], dt, tag="gm")
                nc.sync.dma_start(out=gm[0:IN, :],
                                  in_=bounce_out.rearrange("a b -> b a"))
                nc.sync.dma_start(out=gm[IN:128, :],
                                  in_=bounce_out.rearrange("a b -> b a"))

                # ---------------- per-superblock round 1 -------------------
                accs1 = [[None] * NKS for _ in range(NSUP)]
                y1s = []
                wfs = [[None, None] for _ in range(NSUP)]
                ssqs, m1s = [], []
                for s in range(NSUP):
                    xsl = xds[s][:, :]
                    # fourier planes
                    sh = sb.tile([128, sbc], dt, tag="sh")
                    nc.scalar.activation(sh[:, :], xsl, AF.Sin, scale=0.5)
                    sq2 = sb.tile([128, sbc], dtb if bf16 else dt, tag="sq2")
                    if bf16:
                        nc.gpsimd.tensor_tensor(out=sq2[:, :], in0=sh[:, :],
                                                in1=sh[:, :], op=ALU.mult)
                    else:
                        nc.scalar.square(sq2[:, :], sh[:, :])
                    sh4 = sb.tile([128, sbc], dt, tag="sh4")
                    nc.scalar.activation(sh4[:, :], xsl, AF.Sin, scale=0.25)
                    sq4 = sb.tile([128, sbc], dtb if bf16 else dt, tag="sq4")
                    nc.gpsimd.tensor_tensor(out=sq4[:, :], in0=sh4[:, :],
                                            in1=sh4[:, :], op=ALU.mult)
                    cdup = sb.tile([128, sbc], dtp, tag="cdup")
                    nc.vector.tensor_scalar(out=cdup[:, :], in0=sq2[:, :],
                                            scalar1=-2.0, scalar2=1.0,
                                            op0=ALU.mult, op1=ALU.add)
                    ch2 = sb.tile([128, sbc], dtp if bf16 else dt, tag="ch2")
                    # bf16: ch2 holds 2*cos(x/2) so pk bottom is a plain mult
                    (nc.vector if bf16 else nc.gpsimd).tensor_scalar(
                                            out=ch2[:, :], in0=sq4[:, :],
                                            scalar1=-4.0 if bf16 else -2.0,
                                            scalar2=2.0 if bf16 else 1.0,
                                            op0=ALU.mult, op1=ALU.add)
                    pk = fourp.tile([128, sbc], dtp, tag="pk", name=f"p1_{s}")
                    nc.gpsimd.tensor_copy(out=pk[0:IN, :], in_=cdup[0:IN, :])
                    if bf16:
                        nc.gpsimd.tensor_tensor(out=pk[IN:128, :],
                                                in0=sh[IN:128, :],
                                                in1=ch2[IN:128, :], op=ALU.mult)
                    else:
                        nc.vector.scalar_tensor_tensor(
                            out=pk[IN:128, :], in0=sh[IN:128, :], scalar=2.0,
                            in1=ch2[IN:128, :], op0=ALU.mult, op1=ALU.mult)

                    # wavelet planes
                    for p in range(2):
                        u2 = sb.tile([128, sbc], dtb if bf16 else dt,
                                     tag="u2")
                        nc.scalar.activation(u2[:, :], xsl, AF.Square,
                                             bias=wvt[:, 2 * p + 1:2 * p + 2],
                                             scale=wvt[:, 2 * p:2 * p + 1])
                        ew = sb.tile([128, sbc], dtb if bf16 else dt,
                                     tag="ew")
                        nc.scalar.activation(ew[:, :], u2[:, :], AF.Exp,
                                             scale=-0.5)
                        wf = sb.tile([128, sbc], dtp, tag="wf")
                        if bf16:
                            uw = sb.tile([128, sbc], dtb, tag="uw")
                            nc.vector.tensor_tensor(out=uw[:, :], in0=u2[:, :],
                                                    in1=ew[:, :], op=ALU.mult)
                            (nc.vector if p == 0 else nc.gpsimd).tensor_tensor(
                                out=wf[:, :], in0=uw[:, :], in1=ew[:, :],
                                op=ALU.subtract)
                        else:
                            nc.vector.scalar_tensor_tensor(
                                out=wf[:, :], in0=u2[:, :], scalar=1.0,
                                in1=ew[:, :], op0=ALU.subtract, op1=ALU.mult)
                        wfs[s][p] = wf

                    # mono planes (used in round 2; compute early)
                    ssq = sb.tile([128, sbc], dtr, tag="ssq")
                    nc.scalar.square(ssq[:, :], xsl)
                    m0 = sb.tile([128, sbc], dtr, tag="m0")
                    nc.gpsimd.tensor_copy(out=m0[0:IN, :], in_=xsl[0:IN, :])
                    nc.scalar.square(m0[IN:128, :], xsl[IN:128, :])
                    m1 = sb.tile([128, sbc], dtr, tag="m1")
                    nc.gpsimd.tensor_tensor(out=m1[:, :], in0=m0[:, :],
                                                in1=ssq[:, :], op=ALU.mult)
                    ssqs.append(m0); m1s.append(m1)

                    # round-1 matmuls: fourier chain j=1..8 then wavelet
                    accs = [accp.tile([OUT, KCOL], dt, tag="acc",
                                      name=f"acc1_{s}_{_k}")
                            for _k in range(NKS)]
                    accs1[s] = accs
                    for k in range(NKS):
                        nc.tensor.matmul(accs[k][:, :], lhs(2),
                                         pk[:, k * KCOL:(k + 1) * KCOL],
                                         start=True, stop=False)
                    for j in range(2, 9):
                        pn = fourp.tile([128, sbc], dtp, tag="pk",
                                        name=f"p{j}_{s}")
                        eng = ((nc.gpsimd if j in (4, 6, 7) else nc.vector)
                               if bf16
                               else nc.gpsimd if j in (3, 5, 7) else nc.vector)
                        eng.tensor_tensor(out=pn[:, :], in0=pk[:, :],
                                          in1=cdup[:, :], op=ALU.mult)
                        for k in range(NKS):
                            nc.tensor.matmul(accs[k][:, :], lhs(1 + j),
                                             pn[:, k * KCOL:(k + 1) * KCOL],
                                             start=False, stop=False)
                        pk = pn
                    for p in range(2):
                        for k in range(NKS):
                            nc.tensor.matmul(accs[k][:, :], lhs(10 + p),
                                             wfs[s][p][:, k * KCOL:(k + 1) * KCOL],
                                             start=False, stop=(p == 1))
                    # spill partial sums PSUM -> SBUF (engine copies)
                    y1 = pers.tile([OUT, sbc], dtr, tag=f"y1_{s}", bufs=2)
                    y1s.append(y1)
                    for k in range(NKS):
                        if (s + k) % 2 == 0:
                            nc.scalar.copy(y1[:, k * KCOL:(k + 1) * KCOL],
                                           accs[k][:, :])
                        else:
                            nc.vector.tensor_copy(
                                out=y1[:, k * KCOL:(k + 1) * KCOL],
                                in_=accs[k][:, :])

                if ablate == 1:
                    for s in range(NSUP):
                        nc.sync.dma_start(
                            out=y_d[:, s * sbc:(s + 1) * sbc],
                            in_=y1s[s][:, :].bitcast(dt))
                    continue

                # ---------------- post-collective remix --------------------
                v = pers.tile([128, 24], dt, tag="vecs")
                gmin, gmax, rng_, msk = v[:, 0:1], v[:, 1:2], v[:, 2:3], v[:, 3:4]
                gmin2, gmax2, rng2 = v[:, 4:5], v[:, 5:6], v[:, 6:7]
                rinv, bb, aa = v[:, 7:8], v[:, 8:9], v[:, 9:10]
                b2, b3, a2, a3 = v[:, 10:11], v[:, 11:12], v[:, 12:13], v[:, 13:14]
                ab, a2b, ab2, rstep = (v[:, 14:15], v[:, 15:16], v[:, 16:17],
                                       v[:, 17:18])
                gmr = gm.rearrange("p (r t) -> p t r", t=2)
                nc.vector.tensor_reduce(out=gmin[:, :], in_=gmr[:, 0, :],
                                        axis=AX.X, op=ALU.min)
                nc.vector.tensor_reduce(out=gmax[:, :], in_=gmr[:, 1, :],
                                        axis=AX.X, op=ALU.max)
                nc.vector.tensor_tensor(out=rng_[:, :], in0=gmax[:, :],
                                        in1=gmin[:, :], op=ALU.subtract)
                nc.vector.tensor_scalar(out=msk[:, :], in0=rng_[:, :],
                                        scalar1=1e-8, scalar2=0.5,
                                        op0=ALU.is_lt, op1=ALU.mult)
                nc.vector.tensor_tensor(out=gmin2[:, :], in0=gmin[:, :],
                                        in1=msk[:, :], op=ALU.subtract)
                nc.vector.tensor_tensor(out=gmax2[:, :], in0=gmax[:, :],
                                        in1=msk[:, :], op=ALU.add)
                nc.vector.tensor_tensor(out=rng2[:, :], in0=gmax2[:, :],
                                        in1=gmin2[:, :], op=ALU.subtract)
                nc.vector.reciprocal(out=rinv[:, :], in_=rng2[:, :])
                nc.vector.tensor_scalar_mul(out=bb[:, :], in0=rinv[:, :],
                                            scalar1=8.0)
                nc.vector.scalar_tensor_tensor(out=aa[:, :], in0=gmin2[:, :],
                                               scalar=-1.0, in1=bb[:, :],
                                               op0=ALU.mult, op1=ALU.mult)
                nc.vector.tensor_tensor(out=b2[:, :], in0=bb[:, :], in1=bb[:, :],
                                        op=ALU.mult)
                nc.vector.tensor_tensor(out=b3[:, :], in0=b2[:, :], in1=bb[:, :],
                                        op=ALU.mult)
                nc.vector.tensor_tensor(out=a2[:, :], in0=aa[:, :], in1=aa[:, :],
                                        op=ALU.mult)
                nc.vector.tensor_tensor(out=a3[:, :], in0=a2[:, :], in1=aa[:, :],
                                        op=ALU.mult)
                nc.vector.tensor_tensor(out=ab[:, :], in0=aa[:, :], in1=bb[:, :],
                                        op=ALU.mult)
                nc.vector.tensor_tensor(out=a2b[:, :], in0=a2[:, :], in1=bb[:, :],
                                        op=ALU.mult)
                nc.vector.tensor_tensor(out=ab2[:, :], in0=aa[:, :], in1=b2[:, :],
                                        op=ALU.mult)
                nc.vector.tensor_scalar_mul(out=rstep[:, :], in0=rng2[:, :],
                                            scalar1=0.125)

                # knot biases: kp[0:64, q] = -kappa(2q+1), kp[64:, q] = -kappa(2q+2)
                kp = pers.tile([128, 4], dt, tag="kp")
                for q in range(4):
                    nc.vector.scalar_tensor_tensor(
                        out=kp[0:IN, q:q + 1], in0=rstep[0:IN, :],
                        scalar=-float(2 * q + 1), in1=gmin2[0:IN, :],
                        op0=ALU.mult, op1=ALU.subtract)
                    if 2 * q + 2 <= 7:
                        nc.vector.scalar_tensor_tensor(
                            out=kp[IN:128, q:q + 1], in0=rstep[IN:128, :],
                            scalar=-float(2 * q + 2), in1=gmin2[IN:128, :],
                            op0=ALU.mult, op1=ALU.subtract)

                # dynamic monomial remix cd (degrees 0..3)
                cd = pers.tile([128, 4 * OUT], dt, tag="cd")
                tmp = pers.tile([128, OUT], dt, tag="cdtmp")
                P0, P1 = pwt[:, 0:32], pwt[:, 32:64]
                P2, P3 = pwt[:, 64:96], pwt[:, 96:128]
                cd0, cd1 = cd[:, 0:32], cd[:, 32:64]
                cd2, cd3 = cd[:, 64:96], cd[:, 96:128]
                nc.vector.tensor_scalar(out=cd0, in0=P1, scalar1=aa[:, 0:1],
                                        scalar2=None, op0=ALU.mult)
                nc.vector.tensor_tensor(out=cd0, in0=cd0, in1=P0, op=ALU.add)
                nc.vector.tensor_scalar(out=tmp[:, :], in0=P2, scalar1=a2[:, 0:1],
                                        scalar2=None, op0=ALU.mult)
                nc.vector.tensor_tensor(out=cd0, in0=cd0, in1=tmp[:, :], op=ALU.add)
                nc.vector.tensor_scalar(out=tmp[:, :], in0=P3, scalar1=a3[:, 0:1],
                                        scalar2=None, op0=ALU.mult)
                nc.vector.tensor_tensor(out=cd0, in0=cd0, in1=tmp[:, :], op=ALU.add)
                nc.vector.tensor_scalar(out=cd1, in0=P1, scalar1=bb[:, 0:1],
                                        scalar2=None, op0=ALU.mult)
                nc.vector.tensor_scalar(out=tmp[:, :], in0=P2, scalar1=ab[:, 0:1],
                                        scalar2=2.0, op0=ALU.mult, op1=ALU.mult)
                nc.vector.tensor_tensor(out=cd1, in0=cd1, in1=tmp[:, :], op=ALU.add)
                nc.vector.tensor_scalar(out=tmp[:, :], in0=P3, scalar1=a2b[:, 0:1],
                                        scalar2=3.0, op0=ALU.mult, op1=ALU.mult)
                nc.vector.tensor_tensor(out=cd1, in0=cd1, in1=tmp[:, :], op=ALU.add)
                nc.vector.tensor_scalar(out=cd2, in0=P2, scalar1=b2[:, 0:1],
                                        scalar2=None, op0=ALU.mult)
                nc.vector.tensor_scalar(out=tmp[:, :], in0=P3, scalar1=ab2[:, 0:1],
                                        scalar2=3.0, op0=ALU.mult, op1=ALU.mult)
                nc.vector.tensor_tensor(out=cd2, in0=cd2, in1=tmp[:, :], op=ALU.add)
                nc.vector.tensor_scalar(out=cd3, in0=P3, scalar1=b3[:, 0:1],
                                        scalar2=None, op0=ALU.mult)
                nc.vector.tensor_tensor(out=cwt[0:64, 0:32], in0=cwt[0:64, 0:32],
                                        in1=cd1[0:64, :], op=ALU.add)
                nc.vector.tensor_tensor(out=cwt[64:128, 0:32],
                                        in0=cwt[64:128, 0:32],
                                        in1=cd2[64:128, :], op=ALU.add)
                nc.vector.tensor_tensor(out=cwt[0:64, 32:64], in0=cwt[0:64, 32:64],
                                        in1=cd3[0:64, :], op=ALU.add)
                # rho coefs c12..c15 = RW4 * b^3; c15 bottom = CO + cd0
                for q in range(4):
                    dst = (cwb if bf16 else cwt)[:, 32 * (12 + q):32 * (13 + q)]
                    nc.vector.tensor_scalar(out=dst,
                                            in0=rwt[:, 32 * q:32 * (q + 1)],
                                            scalar1=b3[:, 0:1], scalar2=None,
                                            op0=ALU.mult)
                dst15 = (cwb if bf16 else cwt)[64:128, 32 * 15:32 * 16]
                nc.vector.tensor_tensor(out=dst15, in0=cot[64:128, :],
                                        in1=cd0[64:128, :], op=ALU.add)

                if ablate == 5:
                    for s in range(NSUP):
                        nc.sync.dma_start(
                            out=y_d[:, s * sbc:(s + 1) * sbc],
                            in_=y1s[s][:, :].bitcast(dt))
                    continue

                # ---------------- per-superblock round 2 -------------------
                for s in range(NSUP):
                    xsl = xds[s][:, :]
                    ssq, m1 = ssqs[s], m1s[s]
                    accs = [accp.tile([OUT, KCOL], dt, tag="acc",
                                      name=f"acc2_{s}_{_k}")
                            for _k in range(NKS)]
                    # rho planes
                    r3s = []
                    for q in range([], range(4))[ablate != 4] if False else (
                            [] if ablate == 4 else range(4)):
                        rows = 128 if q < 3 else 64
                        rr = sb.tile([128, sbc], dt, tag="rr")
                        if q < (1 if bf16 else 2):
                            nc.scalar.activation(rr[0:rows, :], xsl[0:rows, :],
                                                 AF.Relu,
                                                 bias=kp[0:rows, q:q + 1])
                        else:
                            nc.gpsimd.tensor_scalar(out=rr[0:rows, :],
                                                    in0=xsl[0:rows, :],
                                                    scalar1=kp[0:rows, q:q + 1],
                                                    scalar2=0.0, op0=ALU.add,
                                                    op1=ALU.max)
                        r2 = sb.tile([128, sbc], dt, tag="r2")
                        eng = nc.gpsimd if q % 2 == 0 else nc.vector
                        eng.tensor_tensor(out=r2[0:rows, :],
                                          in0=rr[0:rows, :],
                                          in1=rr[0:rows, :], op=ALU.mult)
                        r3 = rho7s[s] if q == 3 else sb.tile([128, sbc], dtp,
                                                             tag="r3")
                        reng = nc.vector if q < 2 else nc.gpsimd
                        reng.tensor_tensor(out=r3[0:rows, :],
                                           in0=r2[0:rows, :],
                                           in1=rr[0:rows, :], op=ALU.mult)
                        r3s.append(r3)
                    for k in range(NKS):
                        ck = slice(k * KCOL, (k + 1) * KCOL)
                        acc = accs[k]
                        # mono chunks c0 (m0 = [x; x^2]) and c1
                        nc.tensor.matmul(acc[:, :], lhs(0), f32r(ssq[:, ck]),
                                         start=True, stop=False)
                        nc.tensor.matmul(acc[:, :], lhs(1), f32r(m1[:, ck]),
                                         start=False, stop=(ablate in (2, 4)))
                        # rho chunks c12..c15 (c15 bottom = ones -> bias/CO)
                        if ablate not in (2, 4):
                            for q in range(4):
                                nc.tensor.matmul(acc[:, :], lhs(12 + q),
                                                 r3s[q][:, ck],
                                                 start=False, stop=(q == 3))
                        # merge spilled round-1 partial while leaving PSUM
                        if k == 0:
                            yt = sb.tile([OUT, sbc], dt, tag="yt")
                        nc.vector.tensor_tensor(
                            out=yt[:, k * KCOL:(k + 1) * KCOL], in0=acc[:, :],
                            in1=y1s[s][:, k * KCOL:(k + 1) * KCOL].bitcast(dt),
                            op=ALU.add)
                        if k == NKS - 1:
                            nc.sync.dma_start(
                                out=y_d[:, s * sbc:(s + 1) * sbc], in_=yt[:, :])
    nc.compile()
    return nc


_NC_CACHE = None


def _get_nc():
    global _NC_CACHE
    if _NC_CACHE is None:
        _NC_CACHE = build_nc()
    return _NC_CACHE


def make_in_maps(inputs):
    consts = fold_constants(inputs)
    x = np.ascontiguousarray(np.asarray(inputs["x"], F32))
    in_maps = []
    for c in range(N_CORES):
        m = {"xs": x[c * BS:(c + 1) * BS]}
        m.update(consts)
        in_maps.append(m)
    return in_maps


def kernel(**inputs) -> np.ndarray:
    from concourse.bass_utils import run_bass_kernel_spmd
    nc = _get_nc()
    in_maps = make_in_maps(inputs)
    res = run_bass_kernel_spmd(nc, in_maps, core_ids=list(range(N_CORES)))
    out = np.concatenate([res.results[c]["y"].T for c in range(N_CORES)], axis=0)
    return np.ascontiguousarray(out, dtype=F32)


# revision 9
# speedup vs baseline: 1.9949x; 1.9949x over previous
"""FKANLinear fused kernel for 8 TRN2 NeuronCores (v2).

Differences from v1 (kernel.py):
  - PSUM restructure: per 512-col block, one accumulation chain per round.
    Round 1 (static coefs: fourier c2..c9, wavelet c10/c11) -> spill PSUM->SBUF
    via DMA.  Round 2 (post-collective: mono c0/c1 with dynamic remix, rho
    c12..c15) re-injects the spill with an identity matmul and adds the bias
    via an ones-plane in chunk 15; result DMAs straight PSUM->DRAM.
  - No m0 tile: chunk 0 runs as two half-height matmuls reading xd (x) and
    ssq (x^2) directly.
  - Bias via CO folded into chunk-15 lower-half coefficients (ones plane).
  - Engine rebalance: ACT/DVE/Pool each ~1/3 of elementwise work; rho relu
    on ACT (per-partition bias) / Pool split.
  - Optional bf16 planes (BF16=True) for 2x DVE throughput on mult chains.
"""

import sys
import numpy as np

if "/opt/trn_rl_repo" not in sys.path:
    sys.path.insert(0, "/opt/trn_rl_repo")

N_CORES = 8
B, IN, OUT = 32768, 64, 32
BS = B // N_CORES          # 4096 rows per core
G, P = 8, 3
TAY = 4
JDEG, JA, JB = 4, 1.0, 1.0
CDEG = 4
FREQ = 8
WCH = 4
TEMP = 2.0

F32 = np.float32


# ----------------------------------------------------------------------------
# host-side folding (shared with v1, CO/RW layouts changed)
# ----------------------------------------------------------------------------

def _softplus(z):
    z = np.asarray(z, np.float64)
    return np.log1p(np.exp(-np.abs(z))) + np.maximum(z, 0.0)


def _softmax(z, axis):
    z = np.asarray(z, np.float64)
    m = z.max(axis=axis, keepdims=True)
    e = np.exp(z - m)
    return e / e.sum(axis=axis, keepdims=True)


def _jacobi_mono():
    a, b = JA, JB
    terms = np.zeros((JDEG + 1, 5))
    terms[0, 0] = 1.0
    if JDEG >= 1:
        terms[1, 1] = 0.5 * 2.0 * (a + 1.0) / np.sqrt(2.0)
        terms[1, 0] = 0.5 * (a - b) / np.sqrt(2.0)
    for n in range(2, JDEG + 1):
        k = n - 1
        A1 = 2 * k + a + b
        A2 = 2 * (k + 1) * (k + a + b + 1) * (A1 + 1)
        A4 = 2 * (k + a) * (k + b) * (A1 + 2)
        c_x = (A1 + 1) * (A1 + 2) * A1 / A2
        c_0 = (A1 + 1) * (a * a - b * b) / A2
        Jn = np.zeros(5)
        Jn[1:] += c_x * terms[n - 1][:4]
        Jn += c_0 * terms[n - 1]
        Jn -= (A4 / A2) * terms[n - 2]
        terms[n] = Jn / np.sqrt(n + 1.0)
    return terms


def _cheby_mono():
    T = np.zeros((CDEG + 1, 5))
    T[0, 0] = 1.0
    T[1, 1] = 1.0
    for n in range(2, CDEG + 1):
        shift = np.zeros(5)
        shift[1:] = T[n - 1][:4]
        T[n] = 2.0 * shift - T[n - 2]
    norm = 1.0 / np.sqrt(np.arange(CDEG + 1) + 1.0)
    return T * norm[:, None]


def _bspline_tspace_phi(t):
    grid = np.concatenate([np.zeros(3), np.linspace(0.0, 8.0, G + 1), np.full(3, 8.0)])
    te = t[:, None]
    bases = ((te >= grid[None, :-1]) & (te < grid[None, 1:])).astype(np.float64)
    mask_last = t == grid[-1]
    bases[mask_last, :] = 0.0
    bases[mask_last, -1] = 1.0
    for r in range(1, P + 1):
        ld = np.maximum(grid[r:-1] - grid[:-(r + 1)], 1e-12)
        rd = np.maximum(grid[r + 1:] - grid[1:-r], 1e-12)
        left = (te - grid[None, :-(r + 1)]) / ld[None, :] * bases[:, :-1]
        right = (grid[None, r + 1:] - te) / rd[None, :] * bases[:, 1:]
        bases = left + right
    return bases


def _bspline_truncpow_matrix():
    S = 6000
    t = np.linspace(0.0, 8.0, S)
    t = t + 1e-7
    t = np.clip(t, 0.0, 8.0)
    phi = _bspline_tspace_phi(t)
    Fm = np.zeros((S, 11))
    Fm[:, 0] = 1.0
    Fm[:, 1] = t
    Fm[:, 2] = t * t
    Fm[:, 3] = t ** 3
    for j in range(1, 8):
        Fm[:, 3 + j] = np.maximum(t - j, 0.0) ** 3
    M, _, _, _ = np.linalg.lstsq(Fm, phi, rcond=None)
    return M


def fold_constants(inputs):
    x = inputs["x"]
    base_v = np.asarray(inputs["base_v"], np.float64)
    base_g = np.asarray(inputs["base_g"], np.float64)
    base_bias = np.asarray(inputs["base_bias"], np.float64)
    gains = np.asarray(inputs["gains"], np.float64)
    alpha = float(_softplus(inputs["alpha_logit"]))
    beta = float(_softplus(inputs["beta_logit"]))
    mixw = _softmax(np.asarray(inputs["mix_logits"], np.float64) / TEMP, axis=-1)
    sg = _softplus(gains)

    def ceff(name, f):
        return np.asarray(inputs[name], np.float64) * mixw[..., f:f + 1] * sg[f] * beta

    C_bs = ceff("bspline_coef", 0)
    C_ty = ceff("taylor_coef", 1)
    C_jb = ceff("jacobi_coef", 2)
    C_cb = ceff("cheby_coef", 3)
    C_fr = ceff("fourier_coef", 4)
    C_wv = ceff("wavelet_coef", 5)

    vn = np.sqrt((base_v ** 2).sum(axis=1, keepdims=True))
    Walpha = alpha * base_g * base_v / vn              # (32, 64)
    bias_alpha = alpha * base_bias                      # (32,)

    mono = np.zeros((OUT, IN, 5))
    fac = np.array([1.0, 1.0, 2.0, 6.0])
    mono[:, :, :4] += C_ty / fac[None, None, :]
    mono += np.einsum("oin,nd->oid", C_jb, _jacobi_mono())
    mono += np.einsum("oin,nd->oid", C_cb, _cheby_mono())

    fnorm = 1.0 / np.sqrt(2.0 * FREQ)
    Ccos = C_fr[:, :, :FREQ] * fnorm
    Csin = C_fr[:, :, FREQ:] * fnorm
    Tc = np.zeros((9, 9)); Tc[0, 0] = 1.0; Tc[1, 1] = 1.0
    Uc = np.zeros((9, 9)); Uc[0, 0] = 1.0; Uc[1, 1] = 2.0
    for n in range(2, 9):
        for M_ in (Tc, Uc):
            sh = np.zeros(9); sh[1:] = M_[n - 1][:8]
            M_[n] = 2.0 * sh - M_[n - 2]
    Ccpow = np.einsum("oik,kj->oij", Ccos, Tc[1:9, :])   # (32,64,9)
    Cspow = np.einsum("oik,kj->oij", Csin, Uc[0:8, :])   # (32,64,9)

    a_w = _softplus(np.asarray(inputs["wavelet_scale_logit"], np.float64)) + 1e-6
    inva = 1.0 / a_w
    nshia = -np.asarray(inputs["wavelet_shift"], np.float64) * inva

    M = _bspline_truncpow_matrix()
    CF = np.einsum("oik,mk->oim", C_bs, M)               # (32,64,11)
    P_poly = CF[:, :, :4]
    R_rho = CF[:, :, 4:]                                 # (32,64,7)

    CW = np.zeros((128, 16 * OUT))

    def put(c, half, arr_oi):
        CW[half * 64:(half + 1) * 64, 32 * c:32 * (c + 1)] = arr_oi.T

    put(0, 0, Walpha + mono[:, :, 1])
    put(0, 1, mono[:, :, 2])
    put(1, 0, mono[:, :, 3])
    put(1, 1, mono[:, :, 4])
    for j in range(1, 9):
        put(1 + j, 0, Ccpow[:, :, j])
        put(1 + j, 1, Cspow[:, :, j - 1])
    put(10, 0, C_wv[:, :, 0]); put(10, 1, C_wv[:, :, 1])
    put(11, 0, C_wv[:, :, 2]); put(11, 1, C_wv[:, :, 3])
    # c12..c15 filled on device (rho coefs; c15 bottom = CO + cd0)

    # static const coefs, on rows 64:128 (feed the ones plane of chunk 15)
    CO = np.zeros((128, OUT))
    CO[64:128, :] = (mono[:, :, 0] + Ccpow[:, :, 0]).T
    CO[64, :] += bias_alpha

    PW = np.zeros((128, 4 * OUT))
    for d in range(4):
        PW[0:64, 32 * d:32 * (d + 1)] = P_poly[:, :, d].T
    PW[64:128] = PW[0:64]

    # rho coefs paired per chunk: RW4[:, q] = [R(2q) ; R(2q+1)], q=3: [R6; 0]
    RW4 = np.zeros((128, 4 * OUT))
    for q in range(4):
        RW4[0:64, 32 * q:32 * (q + 1)] = R_rho[:, :, 2 * q].T
        if 2 * q + 1 < 7:
            RW4[64:128, 32 * q:32 * (q + 1)] = R_rho[:, :, 2 * q + 1].T

    WVP = np.zeros((128, 4))
    for p in range(2):
        WVP[0:64, 2 * p] = inva[:, 2 * p]
        WVP[64:128, 2 * p] = inva[:, 2 * p + 1]
        WVP[0:64, 2 * p + 1] = nshia[:, 2 * p]
        WVP[64:128, 2 * p + 1] = nshia[:, 2 * p + 1]

    ID = np.eye(128)

    return {
        "CW": CW.astype(F32), "CO": CO.astype(F32), "PW": PW.astype(F32),
        "RW4": RW4.astype(F32), "WVP": WVP.astype(F32), "ID": ID.astype(F32),
        "ID32": np.eye(OUT).astype(F32),
    }


# ----------------------------------------------------------------------------
# numpy emulation of the device algorithm (validates folding + structure)
# ----------------------------------------------------------------------------

def _rnd(x, bf16):
    if not bf16:
        return np.asarray(x, np.float32)
    import ml_dtypes
    return np.asarray(x, ml_dtypes.bfloat16).astype(np.float32)


def numpy_forward(inputs, bf16=False):
    consts = fold_constants(inputs)
    x = np.asarray(inputs["x"], np.float64)
    CW = consts["CW"].astype(np.float64)
    CO = consts["CO"].astype(np.float64)
    PW = consts["PW"].astype(np.float64)
    RW4 = consts["RW4"].astype(np.float64)
    WVP = consts["WVP"].astype(np.float64)

    xmin = x.min(axis=0); xmax = x.max(axis=0)
    pad = (xmax - xmin) < 1e-8
    xmin = np.where(pad, xmin - 0.5, xmin)
    xmax = np.where(pad, xmax + 0.5, xmax)
    rng = xmax - xmin
    b = 8.0 / rng
    a = -xmin * b
    P_poly = np.stack([PW[0:64, 32 * d:32 * (d + 1)] for d in range(4)], axis=-1)
    binom = {(0, 0): 1, (1, 0): 1, (1, 1): 1, (2, 0): 1, (2, 1): 2, (2, 2): 1,
             (3, 0): 1, (3, 1): 3, (3, 2): 3, (3, 3): 1}
    Cdyn = np.zeros((IN, OUT, 4))
    for d in range(4):
        for e in range(d + 1):
            Cdyn[:, :, e] += P_poly[:, :, d] * (binom[(d, e)] * a ** (d - e) * b ** e)[:, None]
    CW = CW.copy()
    CW[0:64, 0:32] += Cdyn[:, :, 1]
    CW[64:128, 0:32] += Cdyn[:, :, 2]
    CW[0:64, 32:64] += Cdyn[:, :, 3]
    # c12..c15 coefs
    b3 = np.concatenate([b ** 3, b ** 3])               # (128,)
    for q in range(4):
        CW[:, 32 * (12 + q):32 * (13 + q)] = RW4[:, 32 * q:32 * (q + 1)] * b3[:, None]
    CW[64:128, 32 * 15:32 * 16] = CO[64:128] + Cdyn[:, :, 0]

    Bn = x.shape[0]
    kap = [xmin + j * rng / 8.0 for j in range(1, 8)]

    sh = np.sin(0.5 * x); sq2 = sh * sh
    sh4 = np.sin(0.25 * x); sq4 = sh4 * sh4
    cosx = 1.0 - 2.0 * sq2
    ch2 = 1.0 - 2.0 * sq4
    sinx = 2.0 * sh * ch2
    ssq = x * x
    x3 = ssq * x; x4 = ssq * ssq

    y = np.zeros((Bn, OUT))

    def mm(cols, plane, rows):
        nonlocal y
        pl = _rnd(plane, bf16) if rows is None else _rnd(plane, bf16)
        lo, hi = rows
        w = CW[lo:hi, cols]
        y = y + _rnd(pl, False) @ w

    # round 1: fourier c2..c9 (bf16-able), wavelet c10/c11
    pk = np.stack([cosx, sinx], axis=1).reshape(Bn, -1)  # emulate [c; s] halves
    # simpler: build planes explicitly per chunk
    cc, ssn = cosx, sinx
    for j in range(1, 9):
        top = _rnd(cc ** j, bf16)
        bot = _rnd(ssn * cc ** (j - 1), bf16)
        w_top = CW[0:64, 32 * (1 + j):32 * (2 + j)]
        w_bot = CW[64:128, 32 * (1 + j):32 * (2 + j)]
        y = y + top @ w_top + bot @ w_bot
    for p in range(2):
        u0 = x * WVP[None, 0:64, 2 * p] + WVP[None, 0:64, 2 * p + 1]
        u1 = x * WVP[None, 64:128, 2 * p] + WVP[None, 64:128, 2 * p + 1]
        f0 = _rnd((u0 ** 2 - 1) * np.exp(-0.5 * u0 ** 2), bf16)
        f1 = _rnd((u1 ** 2 - 1) * np.exp(-0.5 * u1 ** 2), bf16)
        y = y + f0 @ CW[0:64, 32 * (10 + p):32 * (11 + p)]
        y = y + f1 @ CW[64:128, 32 * (10 + p):32 * (11 + p)]
    # round 2: mono (f32), rho (bf16-able), ones/CO
    y = y + x @ CW[0:64, 0:32] + ssq @ CW[64:128, 0:32]
    y = y + x3 @ CW[0:64, 32:64] + x4 @ CW[64:128, 32:64]
    for q in range(4):
        j0 = 2 * q
        top = _rnd(np.maximum(x - kap[j0][None, :], 0.0) ** 3, bf16)
        y = y + top @ CW[0:64, 32 * (12 + q):32 * (13 + q)]
        if j0 + 1 < 7:
            bot = _rnd(np.maximum(x - kap[j0 + 1][None, :], 0.0) ** 3, bf16)
            y = y + bot @ CW[64:128, 32 * (12 + q):32 * (13 + q)]
    y = y + np.ones((Bn, 64)) @ CW[64:128, 32 * 15:32 * 16]
    return y.astype(F32)


# ----------------------------------------------------------------------------
# device kernel
# ----------------------------------------------------------------------------

SBC = 1024                  # elementwise superblock columns
BF16 = True
BUILD_KW = {"bf16": BF16}


def build_nc(debug=False, reps=1, no_collective=False, sbc=SBC, bf16=BF16,
             ablate=0):
    import concourse.bass as bass
    import concourse.bacc as bacc
    import concourse.mybir as mybir
    import concourse.tile as tile

    dt = mybir.dt.float32
    dtr = mybir.dt.float32r
    dtb = mybir.dt.bfloat16
    dtp = dtb if bf16 else dtr          # plane dtype for matmul rhs
    AF = mybir.ActivationFunctionType
    ALU = mybir.AluOpType
    AX = mybir.AxisListType

    NSUP = BS // sbc
    GRP = 512
    NGS = sbc // GRP            # groups per superblock
    KCOL = 512
    NKS = sbc // KCOL           # col-blocks per superblock

    def f32r(ap):
        return ap if ap.dtype in (dtr, dtb) else ap.bitcast(dtr)

    nc = bacc.Bacc("TRN2", target_bir_lowering=False, debug=False,
                   enable_asserts=True, num_devices=N_CORES)

    xs = nc.dram_tensor("xs", [BS, IN], dt, kind="ExternalInput").ap()
    cw_d = nc.dram_tensor("CW", [128, 16 * OUT], dtr, kind="ExternalInput").ap()
    co_d = nc.dram_tensor("CO", [128, OUT], dt, kind="ExternalInput").ap()
    pw_d = nc.dram_tensor("PW", [128, 4 * OUT], dt, kind="ExternalInput").ap()
    rw_d = nc.dram_tensor("RW4", [128, 4 * OUT], dt, kind="ExternalInput").ap()
    wv_d = nc.dram_tensor("WVP", [128, 4], dt, kind="ExternalInput").ap()
    id_d = nc.dram_tensor("ID", [128, 128], dt, kind="ExternalInput").ap()
    id32_d = nc.dram_tensor("ID32", [OUT, OUT], dtr, kind="ExternalInput").ap()
    y_d = nc.dram_tensor("y", [OUT, BS], dt, kind="ExternalOutput").ap()

    with tile.TileContext(nc) as tc:
        with (
            tc.tile_pool(name="const", bufs=1) as cpool,
            tc.tile_pool(name="sb", bufs=2) as sb,
            tc.tile_pool(name="fourp", bufs=3) as fourp,
            tc.tile_pool(name="xpipe", bufs=3) as xpipe,
            tc.tile_pool(name="pers", bufs=1) as pers,
            tc.tile_pool(name="tp", bufs=2, space="PSUM") as tps,
            tc.tile_pool(name="acc", bufs=4, space="PSUM") as accp,
            tc.tile_pool(name="dram", bufs=1, space="DRAM") as dram,
        ):
            # constants
            cwt = cpool.tile([128, 16 * OUT], dtr, tag="cwt")
            cot = cpool.tile([128, OUT], dt, tag="cot")
            pwt = cpool.tile([128, 4 * OUT], dt, tag="pwt")
            rwt = cpool.tile([128, 4 * OUT], dt, tag="rwt")
            wvt = cpool.tile([128, 4], dt, tag="wvt")
            idt = cpool.tile([128, 128], dt, tag="idt")
            id32 = cpool.tile([OUT, OUT], dtr, tag="id32")
            nc.sync.dma_start(out=cwt[:, :], in_=cw_d[:, :])
            nc.sync.dma_start(out=cot[:, :], in_=co_d[:, :])
            nc.sync.dma_start(out=pwt[:, :], in_=pw_d[:, :])
            nc.sync.dma_start(out=rwt[:, :], in_=rw_d[:, :])
            nc.sync.dma_start(out=wvt[:, :], in_=wv_d[:, :])
            nc.sync.dma_start(out=idt[:, :], in_=id_d[:, :])
            nc.sync.dma_start(out=id32[:, :], in_=id32_d[:, :])

            # bf16 coef tile for bf16 plane chunks (c2..c15)
            if bf16:
                cwb = cpool.tile([128, 16 * OUT], dtb, tag="cwb")
                nc.vector.tensor_copy(out=cwb[:, 64:384],
                                      in_=cwt[:, 64:384].bitcast(dt))

            def lhs(c0, c1=None, rows=(0, 128)):
                """matmul lhsT slice for chunk c0 (cols 32c0..32c0+32)."""
                lo, hi = rows
                t = cwb if (bf16 and c0 >= 2) else cwt
                return f32r(t[lo:hi, 32 * c0:32 * (c0 + 1)])

            # persistent rho7/ones planes per superblock (bottom = 1.0)
            rho7s = []
            ones_sc = cpool.tile([128, sbc], dt, tag="ones_sc")
            nc.vector.memset(ones_sc[64:128, :], 1.0)
            for s in range(NSUP):
                r7 = pers.tile([128, sbc], dtp, tag=f"rho7_{s}")
                if bf16:
                    nc.vector.memset(r7[64:128, :], 1.0)
                else:
                    nc.scalar.copy(r7[64:128, :], ones_sc[64:128, :])
                rho7s.append(r7)

            for _rep in range(reps):
                mm = pers.tile([128, 2 * NSUP], dt, tag="mm")
                xds = []

                # ---------------- phase A: load, transpose, min/max --------
                for s in range(NSUP):
                    xd = pers.tile([128, sbc], dtr, tag=f"xd{s}")
                    xds.append(xd)
                    for g in range(NGS):
                        gg = s * NGS + g
                        base = s * sbc + g * GRP
                        xt8 = xpipe.tile([128, 8 * IN], dt, tag="xin")
                        src = xs[base:base + GRP, :].rearrange(
                            "(t p) i -> p t i", p=128)
                        dst = xt8.rearrange("p (t i) -> p t i", i=2 * IN)
                        nc.sync.dma_start(out=dst[:, :, 0:IN], in_=src)
                        nc.sync.dma_start(out=dst[:, :, IN:2 * IN], in_=src)
                        tp = tps.tile([128, 512], dt, tag="tp")
                        for t in range(4):
                            nc.tensor.transpose(
                                tp[:, 128 * t:128 * (t + 1)],
                                xt8[:, t * 128:(t + 1) * 128], idt[:, :])
                        eng = ((nc.scalar, nc.vector)[gg % 2] if bf16 else
                               (nc.scalar, nc.vector)[gg % 2])
                        if eng is nc.scalar:
                            eng.copy(xd[:, g * GRP:(g + 1) * GRP], tp[:, :])
                        else:
                            eng.tensor_copy(out=xd[:, g * GRP:(g + 1) * GRP],
                                            in_=tp[:, :])
                    nc.vector.tensor_reduce(out=mm[:, s:s + 1], in_=xd[:, :],
                                            axis=AX.X, op=ALU.min)
                    nc.vector.tensor_reduce(out=mm[:, NSUP + s:NSUP + s + 1],
                                            in_=xd[:, :], axis=AX.X, op=ALU.max)

                locmin = pers.tile([128, 1], dt, tag="locmin")
                locmax = pers.tile([128, 1], dt, tag="locmax")
                nc.vector.tensor_reduce(out=locmin[:, :], in_=mm[:, 0:NSUP],
                                        axis=AX.X, op=ALU.min)
                nc.vector.tensor_reduce(out=locmax[:, :], in_=mm[:, NSUP:2 * NSUP],
                                        axis=AX.X, op=ALU.max)

                # ---------------- collective: allgather min/max ------------
                bounce_in = dram.tile([2, IN], dt, tag="cin")
                bounce_out = dram.tile([2 * N_CORES, IN], dt, tag="cout")
                nc.sync.dma_start(out=bounce_in[0:1, :], in_=locmin[0:IN, :])
                nc.sync.dma_start(out=bounce_in[1:2, :], in_=locmax[0:IN, :])
                if no_collective:
                    for r in range(N_CORES):
                        nc.gpsimd.dma_start(out=bounce_out[2 * r:2 * r + 2, :],
                                            in_=bounce_in[:, :])
                else:
                    nc.gpsimd.collective_compute(
                        "AllGather", mybir.AluOpType.bypass,
                        replica_groups=[list(range(N_CORES))],
                        ins=[bounce_in.opt()],
                        outs=[bounce_out.opt()],
                    )
                gm = pers.tile([128, 2 * N_CORES], dt, tag="gm")
                nc.sync.dma_start(out=gm[0:IN, :],
                                  in_=bounce_out.rearrange("a b -> b a"))
                nc.sync.dma_start(out=gm[IN:128, :],
                                  in_=bounce_out.rearrange("a b -> b a"))

                # ---------------- per-superblock round 1 -------------------
                accs1 = [[None] * NKS for _ in range(NSUP)]
                y1s = []
                wfs = [[None, None] for _ in range(NSUP)]
                ssqs, m1s = [], []
                for s in range(NSUP):
                    xsl = xds[s][:, :]
                    # fourier planes
                    sh = sb.tile([128, sbc], dt, tag="sh")
                    nc.scalar.activation(sh[:, :], xsl, AF.Sin, scale=0.5)
                    sq2 = sb.tile([128, sbc], dtb if bf16 else dt, tag="sq2")
                    if bf16:
                        nc.gpsimd.tensor_tensor(out=sq2[:, :], in0=sh[:, :],
                                                in1=sh[:, :], op=ALU.mult)
                    else:
                        nc.scalar.square(sq2[:, :], sh[:, :])
                    sh4 = sb.tile([128, sbc], dt, tag="sh4")
                    nc.scalar.activation(sh4[:, :], xsl, AF.Sin, scale=0.25)
                    sq4 = sb.tile([128, sbc], dtb if bf16 else dt, tag="sq4")
                    nc.gpsimd.tensor_tensor(out=sq4[:, :], in0=sh4[:, :],
                                            in1=sh4[:, :], op=ALU.mult)
                    cdup = sb.tile([128, sbc], dtp, tag="cdup")
                    nc.vector.tensor_scalar(out=cdup[:, :], in0=sq2[:, :],
                                            scalar1=-2.0, scalar2=1.0,
                                            op0=ALU.mult, op1=ALU.add)
                    ch2 = sb.tile([128, sbc], dtp if bf16 else dt, tag="ch2")
                    (nc.vector if bf16 else nc.gpsimd).tensor_scalar(
                                            out=ch2[:, :], in0=sq4[:, :],
                                            scalar1=-2.0, scalar2=1.0,
                                            op0=ALU.mult, op1=ALU.add)
                    pk = fourp.tile([128, sbc], dtp, tag="pk", name=f"p1_{s}")
                    nc.gpsimd.tensor_copy(out=pk[0:IN, :], in_=cdup[0:IN, :])
                    nc.vector.scalar_tensor_tensor(
                        out=pk[IN:128, :], in0=sh[IN:128, :], scalar=2.0,
                        in1=ch2[IN:128, :], op0=ALU.mult, op1=ALU.mult)

                    # wavelet planes
                    for p in range(2):
                        u2 = sb.tile([128, sbc], dtb if bf16 else dt,
                                     tag="u2")
                        nc.scalar.activation(u2[:, :], xsl, AF.Square,
                                             bias=wvt[:, 2 * p + 1:2 * p + 2],
                                             scale=wvt[:, 2 * p:2 * p + 1])
                        ew = sb.tile([128, sbc], dtb if bf16 else dt,
                                     tag="ew")
                        nc.scalar.activation(ew[:, :], u2[:, :], AF.Exp,
                                             scale=-0.5)
                        wf = sb.tile([128, sbc], dtp, tag="wf")
                        weng = nc.vector
                        weng.scalar_tensor_tensor(
                            out=wf[:, :], in0=u2[:, :], scalar=1.0,
                            in1=ew[:, :], op0=ALU.subtract, op1=ALU.mult)
                        wfs[s][p] = wf

                    # mono planes (used in round 2; compute early)
                    ssq = sb.tile([128, sbc], dtr, tag="ssq")
                    nc.scalar.square(ssq[:, :], xsl)
                    m0 = sb.tile([128, sbc], dtr, tag="m0")
                    nc.vector.tensor_copy(out=m0[0:IN, :], in_=xsl[0:IN, :])
                    nc.scalar.square(m0[IN:128, :], xsl[IN:128, :])
                    m1 = sb.tile([128, sbc], dtr, tag="m1")
                    nc.vector.tensor_tensor(out=m1[0:IN, :],
                                                in0=xsl[0:IN, :],
                                                in1=ssq[0:IN, :], op=ALU.mult)
                    nc.scalar.square(m1[IN:128, :], ssq[IN:128, :])
                    ssqs.append(m0); m1s.append(m1)

                    # round-1 matmuls: fourier chain j=1..8 then wavelet
                    accs = [accp.tile([OUT, KCOL], dt, tag="acc",
                                      name=f"acc1_{s}_{_k}")
                            for _k in range(NKS)]
                    accs1[s] = accs
                    for k in range(NKS):
                        nc.tensor.matmul(accs[k][:, :], lhs(2),
                                         pk[:, k * KCOL:(k + 1) * KCOL],
                                         start=True, stop=False)
                    for j in range(2, 9):
                        pn = fourp.tile([128, sbc], dtp, tag="pk",
                                        name=f"p{j}_{s}")
                        eng = ((nc.gpsimd if j in (4, 7) else nc.vector)
                               if bf16
                               else nc.gpsimd if j in (3, 5, 7) else nc.vector)
                        eng.tensor_tensor(out=pn[:, :], in0=pk[:, :],
                                          in1=cdup[:, :], op=ALU.mult)
                        for k in range(NKS):
                            nc.tensor.matmul(accs[k][:, :], lhs(1 + j),
                                             pn[:, k * KCOL:(k + 1) * KCOL],
                                             start=False, stop=False)
                        pk = pn
                    for p in range(2):
                        for k in range(NKS):
                            nc.tensor.matmul(accs[k][:, :], lhs(10 + p),
                                             wfs[s][p][:, k * KCOL:(k + 1) * KCOL],
                                             start=False, stop=(p == 1))
                    # spill partial sums PSUM -> SBUF (engine copies)
                    y1 = pers.tile([OUT, sbc], dtr, tag=f"y1_{s}")
                    y1s.append(y1)
                    for k in range(NKS):
                        nc.scalar.copy(y1[:, k * KCOL:(k + 1) * KCOL],
                                       accs[k][:, :])

                if ablate == 1:
                    for s in range(NSUP):
                        nc.sync.dma_start(
                            out=y_d[:, s * sbc:(s + 1) * sbc],
                            in_=y1s[s][:, :].bitcast(dt))
                    continue

                # ---------------- post-collective remix --------------------
                v = pers.tile([128, 24], dt, tag="vecs")
                gmin, gmax, rng_, msk = v[:, 0:1], v[:, 1:2], v[:, 2:3], v[:, 3:4]
                gmin2, gmax2, rng2 = v[:, 4:5], v[:, 5:6], v[:, 6:7]
                rinv, bb, aa = v[:, 7:8], v[:, 8:9], v[:, 9:10]
                b2, b3, a2, a3 = v[:, 10:11], v[:, 11:12], v[:, 12:13], v[:, 13:14]
                ab, a2b, ab2, rstep = (v[:, 14:15], v[:, 15:16], v[:, 16:17],
                                       v[:, 17:18])
                gmr = gm.rearrange("p (r t) -> p t r", t=2)
                nc.vector.tensor_reduce(out=gmin[:, :], in_=gmr[:, 0, :],
                                        axis=AX.X, op=ALU.min)
                nc.vector.tensor_reduce(out=gmax[:, :], in_=gmr[:, 1, :],
                                        axis=AX.X, op=ALU.max)
                nc.vector.tensor_tensor(out=rng_[:, :], in0=gmax[:, :],
                                        in1=gmin[:, :], op=ALU.subtract)
                nc.vector.tensor_scalar(out=msk[:, :], in0=rng_[:, :],
                                        scalar1=1e-8, scalar2=0.5,
                                        op0=ALU.is_lt, op1=ALU.mult)
                nc.vector.tensor_tensor(out=gmin2[:, :], in0=gmin[:, :],
                                        in1=msk[:, :], op=ALU.subtract)
                nc.vector.tensor_tensor(out=gmax2[:, :], in0=gmax[:, :],
                                        in1=msk[:, :], op=ALU.add)
                nc.vector.tensor_tensor(out=rng2[:, :], in0=gmax2[:, :],
                                        in1=gmin2[:, :], op=ALU.subtract)
                nc.vector.reciprocal(out=rinv[:, :], in_=rng2[:, :])
                nc.vector.tensor_scalar_mul(out=bb[:, :], in0=rinv[:, :],
                                            scalar1=8.0)
                nc.vector.scalar_tensor_tensor(out=aa[:, :], in0=gmin2[:, :],
                                               scalar=-1.0, in1=bb[:, :],
                                               op0=ALU.mult, op1=ALU.mult)
                nc.vector.tensor_tensor(out=b2[:, :], in0=bb[:, :], in1=bb[:, :],
                                        op=ALU.mult)
                nc.vector.tensor_tensor(out=b3[:, :], in0=b2[:, :], in1=bb[:, :],
                                        op=ALU.mult)
                nc.vector.tensor_tensor(out=a2[:, :], in0=aa[:, :], in1=aa[:, :],
                                        op=ALU.mult)
                nc.vector.tensor_tensor(out=a3[:, :], in0=a2[:, :], in1=aa[:, :],
                                        op=ALU.mult)
                nc.vector.tensor_tensor(out=ab[:, :], in0=aa[:, :], in1=bb[:, :],
                                        op=ALU.mult)
                nc.vector.tensor_tensor(out=a2b[:, :], in0=a2[:, :], in1=bb[:, :],
                                        op=ALU.mult)
                nc.vector.tensor_tensor(out=ab2[:, :], in0=aa[:, :], in1=b2[:, :],
                                        op=ALU.mult)
                nc.vector.tensor_scalar_mul(out=rstep[:, :], in0=rng2[:, :],
                                            scalar1=0.125)

                # knot biases: kp[0:64, q] = -kappa(2q+1), kp[64:, q] = -kappa(2q+2)
                kp = pers.tile([128, 4], dt, tag="kp")
                for q in range(4):
                    nc.vector.scalar_tensor_tensor(
                        out=kp[0:IN, q:q + 1], in0=rstep[0:IN, :],
                        scalar=-float(2 * q + 1), in1=gmin2[0:IN, :],
                        op0=ALU.mult, op1=ALU.subtract)
                    if 2 * q + 2 <= 7:
                        nc.vector.scalar_tensor_tensor(
                            out=kp[IN:128, q:q + 1], in0=rstep[IN:128, :],
                            scalar=-float(2 * q + 2), in1=gmin2[IN:128, :],
                            op0=ALU.mult, op1=ALU.subtract)

                # dynamic monomial remix cd (degrees 0..3)
                cd = pers.tile([128, 4 * OUT], dt, tag="cd")
                tmp = pers.tile([128, OUT], dt, tag="cdtmp")
                P0, P1 = pwt[:, 0:32], pwt[:, 32:64]
                P2, P3 = pwt[:, 64:96], pwt[:, 96:128]
                cd0, cd1 = cd[:, 0:32], cd[:, 32:64]
                cd2, cd3 = cd[:, 64:96], cd[:, 96:128]
                nc.vector.tensor_scalar(out=cd0, in0=P1, scalar1=aa[:, 0:1],
                                        scalar2=None, op0=ALU.mult)
                nc.vector.tensor_tensor(out=cd0, in0=cd0, in1=P0, op=ALU.add)
                nc.vector.tensor_scalar(out=tmp[:, :], in0=P2, scalar1=a2[:, 0:1],
                                        scalar2=None, op0=ALU.mult)
                nc.vector.tensor_tensor(out=cd0, in0=cd0, in1=tmp[:, :], op=ALU.add)
                nc.vector.tensor_scalar(out=tmp[:, :], in0=P3, scalar1=a3[:, 0:1],
                                        scalar2=None, op0=ALU.mult)
                nc.vector.tensor_tensor(out=cd0, in0=cd0, in1=tmp[:, :], op=ALU.add)
                nc.vector.tensor_scalar(out=cd1, in0=P1, scalar1=bb[:, 0:1],
                                        scalar2=None, op0=ALU.mult)
                nc.vector.tensor_scalar(out=tmp[:, :], in0=P2, scalar1=ab[:, 0:1],
                                        scalar2=2.0, op0=ALU.mult, op1=ALU.mult)
                nc.vector.tensor_tensor(out=cd1, in0=cd1, in1=tmp[:, :], op=ALU.add)
                nc.vector.tensor_scalar(out=tmp[:, :], in0=P3, scalar1=a2b[:, 0:1],
                                        scalar2=3.0, op0=ALU.mult, op1=ALU.mult)
                nc.vector.tensor_tensor(out=cd1, in0=cd1, in1=tmp[:, :], op=ALU.add)
                nc.vector.tensor_scalar(out=cd2, in0=P2, scalar1=b2[:, 0:1],
                                        scalar2=None, op0=ALU.mult)
                nc.vector.tensor_scalar(out=tmp[:, :], in0=P3, scalar1=ab2[:, 0:1],
                                        scalar2=3.0, op0=ALU.mult, op1=ALU.mult)
                nc.vector.tensor_tensor(out=cd2, in0=cd2, in1=tmp[:, :], op=ALU.add)
                nc.vector.tensor_scalar(out=cd3, in0=P3, scalar1=b3[:, 0:1],
                                        scalar2=None, op0=ALU.mult)
                nc.vector.tensor_tensor(out=cwt[0:64, 0:32], in0=cwt[0:64, 0:32],
                                        in1=cd1[0:64, :], op=ALU.add)
                nc.vector.tensor_tensor(out=cwt[64:128, 0:32],
                                        in0=cwt[64:128, 0:32],
                                        in1=cd2[64:128, :], op=ALU.add)
                nc.vector.tensor_tensor(out=cwt[0:64, 32:64], in0=cwt[0:64, 32:64],
                                        in1=cd3[0:64, :], op=ALU.add)
                # rho coefs c12..c15 = RW4 * b^3; c15 bottom = CO + cd0
                for q in range(4):
                    dst = (cwb if bf16 else cwt)[:, 32 * (12 + q):32 * (13 + q)]
                    nc.vector.tensor_scalar(out=dst,
                                            in0=rwt[:, 32 * q:32 * (q + 1)],
                                            scalar1=b3[:, 0:1], scalar2=None,
                                            op0=ALU.mult)
                dst15 = (cwb if bf16 else cwt)[64:128, 32 * 15:32 * 16]
                nc.vector.tensor_tensor(out=dst15, in0=cot[64:128, :],
                                        in1=cd0[64:128, :], op=ALU.add)

                if ablate == 5:
                    for s in range(NSUP):
                        nc.sync.dma_start(
                            out=y_d[:, s * sbc:(s + 1) * sbc],
                            in_=y1s[s][:, :].bitcast(dt))
                    continue

                # ---------------- per-superblock round 2 -------------------
                for s in range(NSUP):
                    xsl = xds[s][:, :]
                    ssq, m1 = ssqs[s], m1s[s]
                    accs = [accp.tile([OUT, KCOL], dt, tag="acc",
                                      name=f"acc2_{s}_{_k}")
                            for _k in range(NKS)]
                    # rho planes
                    r3s = []
                    for q in range([], range(4))[ablate != 4] if False else (
                            [] if ablate == 4 else range(4)):
                        rows = 128 if q < 3 else 64
                        rr = sb.tile([128, sbc], dt, tag="rr")
                        if q < (1 if bf16 else 2):
                            nc.scalar.activation(rr[0:rows, :], xsl[0:rows, :],
                                                 AF.Relu,
                                                 bias=kp[0:rows, q:q + 1])
                        else:
                            nc.gpsimd.tensor_scalar(out=rr[0:rows, :],
                                                    in0=xsl[0:rows, :],
                                                    scalar1=kp[0:rows, q:q + 1],
                                                    scalar2=0.0, op0=ALU.add,
                                                    op1=ALU.max)
                        r2 = sb.tile([128, sbc], dt, tag="r2")
                        eng = nc.gpsimd if q % 2 == 0 else nc.vector
                        eng.tensor_tensor(out=r2[0:rows, :],
                                          in0=rr[0:rows, :],
                                          in1=rr[0:rows, :], op=ALU.mult)
                        r3 = rho7s[s] if q == 3 else sb.tile([128, sbc], dtp,
                                                             tag="r3")
                        reng = nc.vector if q < 2 else nc.gpsimd
                        reng.tensor_tensor(out=r3[0:rows, :],
                                           in0=r2[0:rows, :],
                                           in1=rr[0:rows, :], op=ALU.mult)
                        r3s.append(r3)
                    for k in range(NKS):
                        ck = slice(k * KCOL, (k + 1) * KCOL)
                        acc = accs[k]
                        # mono chunks c0 (m0 = [x; x^2]) and c1
                        nc.tensor.matmul(acc[:, :], lhs(0), f32r(ssq[:, ck]),
                                         start=True, stop=False)
                        nc.tensor.matmul(acc[:, :], lhs(1), f32r(m1[:, ck]),
                                         start=False, stop=(ablate in (2, 4)))
                        # rho chunks c12..c15 (c15 bottom = ones -> bias/CO)
                        if ablate not in (2, 4):
                            for q in range(4):
                                nc.tensor.matmul(acc[:, :], lhs(12 + q),
                                                 r3s[q][:, ck],
                                                 start=False, stop=(q == 3))
                        # merge spilled round-1 partial while leaving PSUM
                        if k == 0:
                            yt = sb.tile([OUT, sbc], dt, tag="yt")
                        nc.vector.tensor_tensor(
                            out=yt[:, k * KCOL:(k + 1) * KCOL], in0=acc[:, :],
                            in1=y1s[s][:, k * KCOL:(k + 1) * KCOL].bitcast(dt),
                            op=ALU.add)
                        if k == NKS - 1:
                            nc.sync.dma_start(
                                out=y_d[:, s * sbc:(s + 1) * sbc], in_=yt[:, :])
    nc.compile()
    return nc


_NC_CACHE = None


def _get_nc():
    global _NC_CACHE
    if _NC_CACHE is None:
        _NC_CACHE = build_nc()
    return _NC_CACHE


def make_in_maps(inputs):
    consts = fold_constants(inputs)
    x = np.ascontiguousarray(np.asarray(inputs["x"], F32))
    in_maps = []
    for c in range(N_CORES):
        m = {"xs": x[c * BS:(c + 1) * BS]}
        m.update(consts)
        in_maps.append(m)
    return in_maps


def kernel(**inputs) -> np.ndarray:
    from concourse.bass_utils import run_bass_kernel_spmd
    nc = _get_nc()
    in_maps = make_in_maps(inputs)
    res = run_bass_kernel_spmd(nc, in_maps, core_ids=list(range(N_CORES)))
    out = np.concatenate([res.results[c]["y"].T for c in range(N_CORES)], axis=0)
    return np.ascontiguousarray(out, dtype=F32)
